# revision 1
# baseline (speedup 1.0000x reference)
"""Trainium2 Bass kernel for Ernie4.5 attention (B=1, S=2048, HID=4096, H=32,
KVH=8, D=128), tensor-parallel over heads across 8 NeuronCores.

Core i owns q-heads 4i..4i+3, kv-head i, and wo rows [512*i, 512*(i+1)).
Each core computes its partial output [S, HID]; the host sums the 8 partials.

Per-core pipeline (all in transposed [feature, seq] layouts so no on-chip
transposition of activations is ever needed):
  1. qT/kT/vT = (w.T @ hsT-chunks) with weights stationary   -> [D, S] tiles
  2. RoPE on qT/kT via stream_shuffle (even/odd partition swap) + host tables
  3. per head: scoresT[sk,sq] = kT.T @ qT ; probsT = exp(scale*scoresT) with
     causal masking via precomputed 0/1 diag masks; row-sums via ones-matmul;
     outT[d,sq] += v[sk,d].T @ probsT accumulated in PSUM; normalize by recip
     row-sums (gpsimd partition_broadcast + vector mul)
  4. final[sq,hid] = sum_c outT[c].T @ wo[c], DMA'd straight to DRAM
"""

import os
import sys
from contextlib import ExitStack

import numpy as np

for _p in ("/opt/trn_rl_repo",):
    if os.path.isdir(_p) and _p not in sys.path:
        sys.path.append(_p)

import ml_dtypes

import concourse.bass as bass
import concourse.mybir as mybir
import concourse.tile as tile
from concourse import bacc
from concourse.bass_utils import run_bass_kernel_spmd
from concourse.masks import make_identity

P = 128
B, S, HID, H, KVH, D = 1, 2048, 4096, 32, 8, 128
NCORES = 8
HL = H // NCORES          # 4 local q heads
NKT = HID // P            # 32 contraction tiles
NSQ = S // P              # 16 seq blocks
CW = 512                  # seq chunk width
NCH = S // CW             # 4 seq chunks
WOC = 512                 # wo output chunk width
NHC = HID // WOC          # 8 wo output chunks
QKV_COLS = HL * D + 2 * D  # 768 local projection columns
SCALE = float(D) ** -0.5
BASE = 10000.0

F32 = mybir.dt.float32
BF16 = mybir.dt.bfloat16
SWAP_MASK = [i ^ 1 for i in range(32)]

LAST_RESULT = None


def _build(act_dt=BF16, table_dt=F32):
    """Emit the SPMD per-core program. act_dt = matmul operand dtype."""
    nc = bacc.Bacc("TRN2", target_bir_lowering=False, debug=False)

    hsT_d = nc.dram_tensor("hsT", [HID, S], act_dt, kind="ExternalInput").ap()
    wqkv_d = nc.dram_tensor("wqkv", [NKT, P, QKV_COLS], act_dt, kind="ExternalInput").ap()
    wo_d = nc.dram_tensor("wo", [HL, P, NHC, WOC], act_dt, kind="ExternalInput").ap()
    cosT_d = nc.dram_tensor("cosT", [P, S], table_dt, kind="ExternalInput").ap()
    ssinT_d = nc.dram_tensor("ssinT", [P, S], table_dt, kind="ExternalInput").ap()
    dmask_d = nc.dram_tensor("dmask", [P, CW // P, CW], act_dt, kind="ExternalInput").ap()
    out_d = nc.dram_tensor("out", [S, HID], F32, kind="ExternalOutput").ap()

    with tile.TileContext(nc) as tc, ExitStack() as ctx:
        const = ctx.enter_context(tc.tile_pool(name="const", bufs=1))
        wpool = ctx.enter_context(tc.tile_pool(name="wpool", bufs=1))
        tabs = ctx.enter_context(tc.tile_pool(name="tabs", bufs=1))
        res = ctx.enter_context(tc.tile_pool(name="res", bufs=1))
        hst = ctx.enter_context(tc.tile_pool(name="hst", bufs=NKT + 4))
        evq = ctx.enter_context(tc.tile_pool(name="evq", bufs=3))
        rope = ctx.enter_context(tc.tile_pool(name="rope", bufs=3))
        vtmp = ctx.enter_context(tc.tile_pool(name="vtmp", bufs=2))
        probs = ctx.enter_context(tc.tile_pool(name="probs", bufs=8))
        norm = ctx.enter_context(tc.tile_pool(name="norm", bufs=2))
        wow = ctx.enter_context(tc.tile_pool(name="wow", bufs=8))
        outsb = ctx.enter_context(tc.tile_pool(name="outsb", bufs=4))
        psA = ctx.enter_context(tc.tile_pool(name="psA", bufs=2, space="PSUM"))
        psS = ctx.enter_context(tc.tile_pool(name="psS", bufs=2, space="PSUM"))
        psO = ctx.enter_context(tc.tile_pool(name="psO", bufs=2, space="PSUM"))
        psR = ctx.enter_context(tc.tile_pool(name="psR", bufs=1, space="PSUM"))
        psT = ctx.enter_context(tc.tile_pool(name="psT", bufs=1, space="PSUM"))

        ones_t = const.tile([P, 1], act_dt)
        nc.vector.memset(ones_t[:], 1.0)
        ident = const.tile([P, P], F32)
        make_identity(nc, ident[:])
        zbias = const.tile([P, 1], F32)
        nc.vector.memset(zbias[:], 0.0)

        w_all = wpool.tile([P, NKT, QKV_COLS], act_dt)
        for k in range(NKT):
            nc.sync.dma_start(w_all[:, k, :], wqkv_d[k, :, :])
        cosT = tabs.tile([P, S], table_dt)
        nc.sync.dma_start(cosT[:], cosT_d[:, :])
        ssinT = tabs.tile([P, S], table_dt)
        nc.sync.dma_start(ssinT[:], ssinT_d[:, :])
        dmask = tabs.tile([P, CW // P, CW], act_dt)
        nc.sync.dma_start(dmask[:], dmask_d[:, :, :])

        # resident activations: qT (4 heads) + kT in one tile; v natural; outT
        qkT = res.tile([P, HL + 1, S], act_dt)
        v_sb = res.tile([P, NSQ, P], act_dt)
        outT = res.tile([P, HL, S], act_dt)

        # ---- phase 1: projections + RoPE + v transpose ----
        for j in range(NCH):
            jsl = bass.ts(j, CW)
            hst_tiles = []
            for k in range(NKT):
                t = hst.tile([P, CW], act_dt, tag="hst")
                nc.sync.dma_start(t[:], hsT_d[k * P:(k + 1) * P, jsl])
                hst_tiles.append(t)
            for c in range(HL + 2):
                ps = psA.tile([P, CW], F32, tag="acc")
                for k in range(NKT):
                    nc.tensor.matmul(
                        ps[:], w_all[:, k, c * P:(c + 1) * P], hst_tiles[k][:],
                        start=(k == 0), stop=(k == NKT - 1))
                if c < HL + 1:  # q heads and k: RoPE then store
                    raw = evq.tile([P, CW], act_dt, tag="raw")
                    nc.scalar.copy(raw[:], ps[:])
                    t1 = rope.tile([P, CW], act_dt, tag="t1")
                    nc.vector.tensor_mul(t1[:], raw[:], cosT[:, jsl])
                    t2 = rope.tile([P, CW], act_dt, tag="t2")
                    nc.vector.stream_shuffle(t2[:], raw[:], SWAP_MASK)
                    t3 = rope.tile([P, CW], act_dt, tag="t3")
                    nc.vector.tensor_mul(t3[:], t2[:], ssinT[:, jsl])
                    nc.vector.tensor_add(qkT[:, c, jsl], t1[:], t3[:])
                else:  # v: evict then PE-transpose into natural layout
                    vt = vtmp.tile([P, CW], F32, tag="vt")
                    nc.scalar.copy(vt[:], ps[:])
                    for b in range(CW // P):
                        pt = psT.tile([P, P], F32, tag="tr")
                        nc.tensor.transpose(pt[:], vt[:, b * P:(b + 1) * P], ident[:])
                        nc.vector.tensor_copy(v_sb[:, j * (CW // P) + b, :], pt[:])

        # ---- phase 2: attention (scoresT layout; causal) ----
        for h in range(HL):
            for j in range(NCH):
                jsl = bass.ts(j, CW)
                nblk = (j + 1) * (CW // P)
                po = psO.tile([P, CW], F32, tag="po")
                pr = psR.tile([1, CW], F32, tag="pr")
                for sk in range(nblk):
                    pss = psS.tile([P, CW], F32, tag="sc")
                    nc.tensor.matmul(
                        pss[:], qkT[:, HL, sk * P:(sk + 1) * P], qkT[:, h, jsl],
                        start=True, stop=True)
                    pb = probs.tile([P, CW], act_dt, tag="pb")
                    nc.scalar.activation(
                        pb[:], pss[:], mybir.ActivationFunctionType.Exp,
                        bias=zbias[:], scale=SCALE)
                    t = sk - j * (CW // P)
                    if t >= 0:  # diagonal block: zero out sq < sk entries
                        nc.vector.tensor_mul(pb[:], pb[:], dmask[:, t, :])
                    nc.tensor.matmul(po[:], v_sb[:, sk, :], pb[:],
                                     start=(sk == 0), stop=(sk == nblk - 1))
                    nc.tensor.matmul(pr[:], ones_t[:], pb[:],
                                     start=(sk == 0), stop=(sk == nblk - 1))
                rc = norm.tile([1, CW], F32, tag="rc")
                nc.vector.reciprocal(rc[:], pr[:])
                rb = norm.tile([P, CW], F32, tag="rb")
                nc.gpsimd.partition_broadcast(rb[:], rc[:], channels=P)
                nc.vector.tensor_mul(outT[:, h, jsl], po[:], rb[:])

        # ---- phase 3: wo projection ----
        for hc in range(NHC):
            wts = []
            for c in range(HL):
                wt = wow.tile([P, WOC], act_dt, tag="wt")
                nc.sync.dma_start(wt[:], wo_d[c, :, hc, :])
                wts.append(wt)
            for sq in range(NSQ):
                pf = psA.tile([P, WOC], F32, tag="acc")
                for c in range(HL):
                    nc.tensor.matmul(pf[:], outT[:, c, sq * P:(sq + 1) * P],
                                     wts[c][:], start=(c == 0), stop=(c == HL - 1))
                ob = outsb.tile([P, WOC], F32, tag="ob")
                nc.scalar.copy(ob[:], pf[:])
                nc.sync.dma_start(out_d[sq * P:(sq + 1) * P, hc * WOC:(hc + 1) * WOC], ob[:])

    nc.compile()
    return nc


def _rope_tables():
    inv_freq = (1.0 / (BASE ** (np.arange(0, D, 2, dtype=np.float32) / D))).astype(np.float32)
    pos = np.arange(S, dtype=np.float32)[:, None]
    ang = pos * inv_freq[None, :]              # [S, D/2]
    sin = np.sin(ang).astype(np.float32).T     # [D/2, S]
    cos = np.cos(ang).astype(np.float32).T
    cosT = np.empty((D, S), np.float32)
    cosT[0::2] = cos
    cosT[1::2] = cos
    ssinT = np.empty((D, S), np.float32)
    ssinT[0::2] = -sin
    ssinT[1::2] = sin
    return cosT, ssinT


def _diag_masks():
    # dmask[p, t, f] = 1 where the scoreT element (sk=128t+p, sq=f) is causal-valid
    p = np.arange(P)[:, None, None]
    t = np.arange(CW // P)[None, :, None]
    f = np.arange(CW)[None, None, :]
    return (f >= P * t + p).astype(np.float32)


_NC_CACHE = {}


def kernel(hidden_states, wq, wk, wv, wo):
    global LAST_RESULT
    act_np = ml_dtypes.bfloat16
    key = "bf16"
    if key not in _NC_CACHE:
        _NC_CACHE[key] = _build()
    nc = _NC_CACHE[key]

    hs = np.asarray(hidden_states, np.float32).reshape(S, HID)
    hsT = np.ascontiguousarray(hs.T).astype(act_np)
    cosT, ssinT = _rope_tables()
    dmask = _diag_masks().astype(act_np)

    in_maps = []
    for i in range(NCORES):
        wqkv = np.concatenate(
            [np.asarray(wq, np.float32)[:, i * HL * D:(i + 1) * HL * D],
             np.asarray(wk, np.float32)[:, i * D:(i + 1) * D],
             np.asarray(wv, np.float32)[:, i * D:(i + 1) * D]], axis=1)
        wqkv = np.ascontiguousarray(wqkv.reshape(NKT, P, QKV_COLS)).astype(act_np)
        wo_i = np.ascontiguousarray(
            np.asarray(wo, np.float32)[i * HL * D:(i + 1) * HL * D, :]
            .reshape(HL, P, NHC, WOC)).astype(act_np)
        in_maps.append({
            "hsT": hsT, "wqkv": wqkv, "wo": wo_i,
            "cosT": cosT, "ssinT": ssinT, "dmask": dmask,
        })

    trace = bool(os.environ.get("BASS_KERNEL_TRACE"))
    res = run_bass_kernel_spmd(nc, in_maps, list(range(NCORES)),
                               trace=trace, trace_cores=[0] if trace else None)
    LAST_RESULT = res
    acc = np.zeros((S, HID), np.float32)
    for i in range(NCORES):
        acc += np.asarray(res.results[i]["out"], np.float32)
    return acc.reshape(B, S, HID)


# revision 6
# speedup vs baseline: 1.1751x; 1.1751x over previous
"""Trainium2 Bass kernel for Ernie4.5 attention (B=1, S=2048, HID=4096, H=32,
KVH=8, D=128), tensor-parallel over heads across 8 NeuronCores.

Core i owns q-heads 4i..4i+3, kv-head i, and wo rows [512*i, 512*(i+1)).
Each core computes its partial output [S, HID]; the host sums the 8 partials.

Per-core pipeline (all in transposed [feature, seq] layouts so no on-chip
transposition of activations is ever needed):
  1. qT/kT/vT = (w.T @ hsT-chunks) with weights stationary   -> [D, S] tiles
  2. RoPE on qT/kT via stream_shuffle (even/odd partition swap) + host tables
  3. per head: scoresT[sk,sq] = kT.T @ qT ; probsT = exp(scale*scoresT) with
     causal masking via precomputed 0/1 diag masks; row-sums via ones-matmul;
     outT[d,sq] += v[sk,d].T @ probsT accumulated in PSUM; normalize by recip
     row-sums (gpsimd partition_broadcast + vector mul)
  4. final[sq,hid] = sum_c outT[c].T @ wo[c], DMA'd straight to DRAM
"""

import os
import sys
from contextlib import ExitStack

import numpy as np

for _p in ("/opt/trn_rl_repo",):
    if os.path.isdir(_p) and _p not in sys.path:
        sys.path.append(_p)

import ml_dtypes

import concourse.bass as bass
import concourse.mybir as mybir
import concourse.tile as tile
from concourse import bacc
from concourse.bass_utils import run_bass_kernel_spmd
from concourse.masks import make_identity

P = 128
B, S, HID, H, KVH, D = 1, 2048, 4096, 32, 8, 128
NCORES = 8
HL = H // NCORES          # 4 local q heads
NKT = HID // P            # 32 contraction tiles
NSQ = S // P              # 16 seq blocks
CW = 512                  # seq chunk width
NCH = S // CW             # 4 seq chunks
KP = 4                    # hsT k-tiles packed per DMA
WOC = 512                 # wo output chunk width
NHC = HID // WOC          # 8 wo output chunks
NCB = HL + 2              # 6 projection column blocks (4 q heads, k, v)
SCALE = float(D) ** -0.5
BASE = 10000.0

F32 = mybir.dt.float32
BF16 = mybir.dt.bfloat16
SWAP_MASK = [i ^ 1 for i in range(32)]

LAST_RESULT = None


def _build(act_dt=BF16, table_dt=F32):
    """Emit the SPMD per-core program. act_dt = matmul operand dtype."""
    nc = bacc.Bacc("TRN2", target_bir_lowering=False, debug=False)

    hsT_d = nc.dram_tensor("hsT", [HID, S], act_dt, kind="ExternalInput").ap()
    wqkv_d = nc.dram_tensor("wqkv", [NCB, P, NKT * P], act_dt, kind="ExternalInput").ap()
    wo_d = nc.dram_tensor("wo", [HL, P, NHC, WOC], act_dt, kind="ExternalInput").ap()
    cosT_d = nc.dram_tensor("cosT", [P, S], table_dt, kind="ExternalInput").ap()
    ssinT_d = nc.dram_tensor("ssinT", [P, S], table_dt, kind="ExternalInput").ap()
    dmask_d = nc.dram_tensor("dmask", [P, CW // P, CW], act_dt, kind="ExternalInput").ap()
    out_d = nc.dram_tensor("out", [S, HID], F32, kind="ExternalOutput").ap()

    with tile.TileContext(nc) as tc, ExitStack() as ctx:
        const = ctx.enter_context(tc.tile_pool(name="const", bufs=1))
        wpool = ctx.enter_context(tc.tile_pool(name="wpool", bufs=1))
        tabs = ctx.enter_context(tc.tile_pool(name="tabs", bufs=1))
        res = ctx.enter_context(tc.tile_pool(name="res", bufs=1))
        hst = ctx.enter_context(tc.tile_pool(name="hst", bufs=NKT // KP + 1))
        evq = ctx.enter_context(tc.tile_pool(name="evq", bufs=3))
        rope = ctx.enter_context(tc.tile_pool(name="rope", bufs=3))
        vtmp = ctx.enter_context(tc.tile_pool(name="vtmp", bufs=2))
        probs = ctx.enter_context(tc.tile_pool(name="probs", bufs=8))
        norm = ctx.enter_context(tc.tile_pool(name="norm", bufs=2))
        wow = ctx.enter_context(tc.tile_pool(name="wow", bufs=8))
        outsb = ctx.enter_context(tc.tile_pool(name="outsb", bufs=4))
        # PSUM: 8 banks total. "big" [128,512]f32 slots shared (same tag)
        # across proj-acc, scores, and wo-acc; psO holds attention outT
        # accumulators; psM shared by v-transpose and rowsums.
        psB = ctx.enter_context(tc.tile_pool(name="psB", bufs=4, space="PSUM"))
        psO = ctx.enter_context(tc.tile_pool(name="psO", bufs=2, space="PSUM"))
        psM = ctx.enter_context(tc.tile_pool(name="psM", bufs=2, space="PSUM"))

        ones_t = const.tile([P, 1], act_dt)
        nc.vector.memset(ones_t[:], 1.0)
        ident = const.tile([P, P], F32)
        make_identity(nc, ident[:])
        zbias = const.tile([P, 1], F32)
        nc.vector.memset(zbias[:], 0.0)

        # weights: one resident tile + one DMA per column block. c=0 first so
        # the first matmuls' inputs land early; the rest follow the first hsT
        # chunk in queue order.
        w_all = wpool.tile([P, NCB, NKT * P], act_dt)

        def _load_wc(c):
            nc.sync.dma_start(w_all[:, c, :], wqkv_d[c, :, :])

        _load_wc(0)

        hst_tiles = {}

        def _load_hst_chunk(j):
            packs = []
            for g in range(NKT // KP):
                t = hst.tile([P, KP, CW], act_dt, tag="hst")
                nc.sync.dma_start(
                    t[:], hsT_d.rearrange("(g kp p) s -> g p kp s", g=NKT // KP, kp=KP, p=P)[
                        g, :, :, bass.ts(j, CW)])
                packs.append(t)
            hst_tiles[j] = packs

        _load_hst_chunk(0)
        for c in range(1, NCB):
            _load_wc(c)

        cosT = tabs.tile([P, S], table_dt)
        nc.sync.dma_start(cosT[:], cosT_d[:, :])
        ssinT = tabs.tile([P, S], table_dt)
        nc.sync.dma_start(ssinT[:], ssinT_d[:, :])
        dmask = tabs.tile([P, CW // P, CW], act_dt)
        nc.sync.dma_start(dmask[:], dmask_d[:, :, :])

        # resident activations: qT (4 heads) + kT in one tile; v natural; outT
        qkT = res.tile([P, HL + 1, S], act_dt)
        v_sb = res.tile([P, NSQ, P], act_dt)
        outT = res.tile([P, HL, S], act_dt)

        # ---- phase 1: projections + RoPE + v transpose ----
        for j in range(NCH):
            jsl = bass.ts(j, CW)
            if j + 1 < NCH:
                _load_hst_chunk(j + 1)
            packs = hst_tiles.pop(j)
            for c in range(NCB):
                ps = psB.tile([P, CW], F32, tag="acc")
                for k in range(NKT):
                    nc.tensor.matmul(
                        ps[:], w_all[:, c, k * P:(k + 1) * P],
                        packs[k // KP][:, k % KP, :],
                        start=(k == 0), stop=(k == NKT - 1))
                if c < HL + 1:  # q heads and k: RoPE then store
                    raw = evq.tile([P, CW], act_dt, tag="raw")
                    nc.scalar.copy(raw[:], ps[:])
                    t1 = rope.tile([P, CW], act_dt, tag="t1")
                    nc.vector.tensor_mul(t1[:], raw[:], cosT[:, jsl])
                    t2 = rope.tile([P, CW], act_dt, tag="t2")
                    nc.vector.stream_shuffle(t2[:], raw[:], SWAP_MASK)
                    t3 = rope.tile([P, CW], act_dt, tag="t3")
                    nc.vector.tensor_mul(t3[:], t2[:], ssinT[:, jsl])
                    nc.vector.tensor_add(qkT[:, c, jsl], t1[:], t3[:])
                else:  # v: evict then PE-transpose into natural layout
                    vt = vtmp.tile([P, CW], F32, tag="vt")
                    nc.scalar.copy(vt[:], ps[:])
                    for b in range(CW // P):
                        pt = psM.tile([P, P], F32, tag="m")
                        nc.tensor.transpose(pt[:], vt[:, b * P:(b + 1) * P], ident[:])
                        nc.vector.tensor_copy(v_sb[:, j * (CW // P) + b, :], pt[:])

        # ---- phase 2: attention (scoresT layout; causal) ----
        for h in range(HL):
            for j in range(NCH):
                jsl = bass.ts(j, CW)
                nblk = (j + 1) * (CW // P)
                po = psO.tile([P, CW], F32, tag="po")
                pr = psM.tile([1, CW], F32, tag="m")
                for sk in range(nblk):
                    pss = psB.tile([P, CW], F32, tag="acc")
                    nc.tensor.matmul(
                        pss[:], qkT[:, HL, sk * P:(sk + 1) * P], qkT[:, h, jsl],
                        start=True, stop=True)
                    pb = probs.tile([P, CW], act_dt, tag="pb")
                    nc.scalar.activation(
                        pb[:], pss[:], mybir.ActivationFunctionType.Exp,
                        bias=zbias[:], scale=SCALE)
                    t = sk - j * (CW // P)
                    if t >= 0:  # diagonal block: zero out sq < sk entries
                        nc.vector.tensor_mul(pb[:], pb[:], dmask[:, t, :])
                    nc.tensor.matmul(po[:], v_sb[:, sk, :], pb[:],
                                     start=(sk == 0), stop=(sk == nblk - 1))
                    nc.tensor.matmul(pr[:], ones_t[:], pb[:],
                                     start=(sk == 0), stop=(sk == nblk - 1))
                # free the rowsum PSUM bank fast, then normalize off PE path
                rs = norm.tile([1, CW], F32, tag="rs")
                nc.scalar.copy(rs[:], pr[:])
                rc = norm.tile([1, CW], F32, tag="rc")
                sc = norm.tile([1, CW], F32, tag="sc")
                nc.vector.reciprocal_approx_accurate(rc[:], rs[:], sc[:])
                rb = norm.tile([P, CW], F32, tag="rb")
                nc.gpsimd.partition_broadcast(rb[:], rc[:], channels=P)
                nc.vector.tensor_mul(outT[:, h, jsl], po[:], rb[:])

        # ---- phase 3: wo projection ----
        for hc in range(NHC):
            wts = []
            for c in range(HL):
                wt = wow.tile([P, WOC], act_dt, tag="wt")
                nc.sync.dma_start(wt[:], wo_d[c, :, hc, :])
                wts.append(wt)
            for sq in range(NSQ):
                pf = psB.tile([P, WOC], F32, tag="acc")
                for c in range(HL):
                    nc.tensor.matmul(pf[:], outT[:, c, sq * P:(sq + 1) * P],
                                     wts[c][:], start=(c == 0), stop=(c == HL - 1))
                ob = outsb.tile([P, WOC], F32, tag="ob")
                nc.vector.tensor_copy(ob[:], pf[:])
                nc.gpsimd.dma_start(out_d[sq * P:(sq + 1) * P, hc * WOC:(hc + 1) * WOC], ob[:])

    nc.compile()
    return nc


def _rope_tables():
    inv_freq = (1.0 / (BASE ** (np.arange(0, D, 2, dtype=np.float32) / D))).astype(np.float32)
    pos = np.arange(S, dtype=np.float32)[:, None]
    ang = pos * inv_freq[None, :]              # [S, D/2]
    sin = np.sin(ang).astype(np.float32).T     # [D/2, S]
    cos = np.cos(ang).astype(np.float32).T
    cosT = np.empty((D, S), np.float32)
    cosT[0::2] = cos
    cosT[1::2] = cos
    ssinT = np.empty((D, S), np.float32)
    ssinT[0::2] = -sin
    ssinT[1::2] = sin
    return cosT, ssinT


def _diag_masks():
    # dmask[p, t, f] = 1 where the scoreT element (sk=128t+p, sq=f) is causal-valid
    p = np.arange(P)[:, None, None]
    t = np.arange(CW // P)[None, :, None]
    f = np.arange(CW)[None, None, :]
    return (f >= P * t + p).astype(np.float32)


_NC_CACHE = {}


def kernel(hidden_states, wq, wk, wv, wo):
    global LAST_RESULT
    act_np = ml_dtypes.bfloat16
    key = "bf16"
    if key not in _NC_CACHE:
        _NC_CACHE[key] = _build()
    nc = _NC_CACHE[key]

    hs = np.asarray(hidden_states, np.float32).reshape(S, HID)
    hsT = np.ascontiguousarray(hs.T).astype(act_np)
    cosT, ssinT = _rope_tables()
    dmask = _diag_masks().astype(act_np)

    in_maps = []
    for i in range(NCORES):
        wqkv = np.concatenate(
            [np.asarray(wq, np.float32)[:, i * HL * D:(i + 1) * HL * D],
             np.asarray(wk, np.float32)[:, i * D:(i + 1) * D],
             np.asarray(wv, np.float32)[:, i * D:(i + 1) * D]], axis=1)
        # [HID, 768] -> [NCB, P, NKT*P]: block c, hid-in-tile p, (k-tile, col)
        wqkv = np.ascontiguousarray(
            wqkv.reshape(NKT, P, NCB, P).transpose(2, 1, 0, 3).reshape(NCB, P, NKT * P)
        ).astype(act_np)
        wo_i = np.ascontiguousarray(
            np.asarray(wo, np.float32)[i * HL * D:(i + 1) * HL * D, :]
            .reshape(HL, P, NHC, WOC)).astype(act_np)
        in_maps.append({
            "hsT": hsT, "wqkv": wqkv, "wo": wo_i,
            "cosT": cosT, "ssinT": ssinT, "dmask": dmask,
        })

    trace = bool(os.environ.get("BASS_KERNEL_TRACE"))
    res = run_bass_kernel_spmd(nc, in_maps, list(range(NCORES)),
                               trace=trace, trace_cores=[0] if trace else None)
    LAST_RESULT = res
    acc = np.zeros((S, HID), np.float32)
    for i in range(NCORES):
        acc += np.asarray(res.results[i]["out"], np.float32)
    return acc.reshape(B, S, HID)


# revision 8
# speedup vs baseline: 1.1990x; 1.0203x over previous
"""Trainium2 Bass kernel for Ernie4.5 attention (B=1, S=2048, HID=4096, H=32,
KVH=8, D=128), tensor-parallel over heads across 8 NeuronCores.

Core i owns q-heads 4i..4i+3, kv-head i, and wo rows [512*i, 512*(i+1)).
Each core computes its partial output [S, HID]; the host sums the 8 partials.

Per-core pipeline (all in transposed [feature, seq] layouts so no on-chip
transposition of activations is ever needed):
  1. qT/kT/vT = (w.T @ hsT-chunks) with weights stationary   -> [D, S] tiles
  2. RoPE on qT/kT via stream_shuffle (even/odd partition swap) + host tables
  3. per head: scoresT[sk,sq] = kT.T @ qT ; probsT = exp(scale*scoresT) with
     causal masking via precomputed 0/1 diag masks; row-sums via ones-matmul;
     outT[d,sq] += v[sk,d].T @ probsT accumulated in PSUM; normalize by recip
     row-sums (gpsimd partition_broadcast + vector mul)
  4. final[sq,hid] = sum_c outT[c].T @ wo[c], DMA'd straight to DRAM
"""

import os
import sys
from contextlib import ExitStack

import numpy as np

for _p in ("/opt/trn_rl_repo",):
    if os.path.isdir(_p) and _p not in sys.path:
        sys.path.append(_p)

import ml_dtypes

import concourse.bass as bass
import concourse.mybir as mybir
import concourse.tile as tile
from concourse import bacc
from concourse.bass_utils import run_bass_kernel_spmd
from concourse.masks import make_identity

P = 128
B, S, HID, H, KVH, D = 1, 2048, 4096, 32, 8, 128
NCORES = 8
HL = H // NCORES          # 4 local q heads
NKT = HID // P            # 32 contraction tiles
NSQ = S // P              # 16 seq blocks
CW = 512                  # seq chunk width
NCH = S // CW             # 4 seq chunks
KP = 4                    # hsT k-tiles packed per DMA
WOC = 512                 # wo output chunk width
NHC = HID // WOC          # 8 wo output chunks
NCB = HL + 2              # 6 projection column blocks (4 q heads, k, v)
SCALE = float(D) ** -0.5
BASE = 10000.0

F32 = mybir.dt.float32
BF16 = mybir.dt.bfloat16
SWAP_MASK = [i ^ 1 for i in range(32)]

LAST_RESULT = None


def _build(act_dt=BF16, table_dt=F32):
    """Emit the SPMD per-core program. act_dt = matmul operand dtype."""
    nc = bacc.Bacc("TRN2", target_bir_lowering=False, debug=False)

    hsT_d = nc.dram_tensor("hsT", [HID, S], act_dt, kind="ExternalInput").ap()
    wqkv_d = nc.dram_tensor("wqkv", [NCB, P, NKT * P], act_dt, kind="ExternalInput").ap()
    wo_d = nc.dram_tensor("wo", [HL, P, NHC, WOC], act_dt, kind="ExternalInput").ap()
    cosT_d = nc.dram_tensor("cosT", [P, S], table_dt, kind="ExternalInput").ap()
    ssinT_d = nc.dram_tensor("ssinT", [P, S], table_dt, kind="ExternalInput").ap()
    dmask_d = nc.dram_tensor("dmask", [P, CW // P, CW], act_dt, kind="ExternalInput").ap()
    out_d = nc.dram_tensor("out", [S, HID], F32, kind="ExternalOutput").ap()

    with tile.TileContext(nc) as tc, ExitStack() as ctx:
        const = ctx.enter_context(tc.tile_pool(name="const", bufs=1))
        wpool = ctx.enter_context(tc.tile_pool(name="wpool", bufs=1))
        tabs = ctx.enter_context(tc.tile_pool(name="tabs", bufs=1))
        res = ctx.enter_context(tc.tile_pool(name="res", bufs=1))
        hst = ctx.enter_context(tc.tile_pool(name="hst", bufs=NKT // KP + 1))
        evq = ctx.enter_context(tc.tile_pool(name="evq", bufs=3))
        rope = ctx.enter_context(tc.tile_pool(name="rope", bufs=3))
        vtmp = ctx.enter_context(tc.tile_pool(name="vtmp", bufs=2))
        probs = ctx.enter_context(tc.tile_pool(name="probs", bufs=8))
        norm = ctx.enter_context(tc.tile_pool(name="norm", bufs=2))
        wow = ctx.enter_context(tc.tile_pool(name="wow", bufs=8))
        outsb = ctx.enter_context(tc.tile_pool(name="outsb", bufs=4))
        # PSUM: 8 banks total. "big" [128,512]f32 slots shared (same tag)
        # across proj-acc, scores, and wo-acc; psO holds attention outT
        # accumulators; psM shared by v-transpose and rowsums.
        psB = ctx.enter_context(tc.tile_pool(name="psB", bufs=4, space="PSUM"))
        psO = ctx.enter_context(tc.tile_pool(name="psO", bufs=2, space="PSUM"))
        psM = ctx.enter_context(tc.tile_pool(name="psM", bufs=2, space="PSUM"))

        ones_t = const.tile([P, 1], act_dt)
        nc.vector.memset(ones_t[:], 1.0)
        ident = const.tile([P, P], F32)
        make_identity(nc, ident[:])
        zbias = const.tile([P, 1], F32)
        nc.vector.memset(zbias[:], 0.0)

        # weights: one resident tile + one DMA per column block. c=0 first so
        # the first matmuls' inputs land early; the rest follow the first hsT
        # chunk in queue order.
        w_all = wpool.tile([P, NCB, NKT * P], act_dt)

        def _load_wc(c):
            nc.sync.dma_start(w_all[:, c, :], wqkv_d[c, :, :])

        _load_wc(0)

        hst_tiles = {}

        def _load_hst_chunk(j):
            packs = []
            for g in range(NKT // KP):
                t = hst.tile([P, KP, CW], act_dt, tag="hst")
                nc.sync.dma_start(
                    t[:], hsT_d.rearrange("(g kp p) s -> g p kp s", g=NKT // KP, kp=KP, p=P)[
                        g, :, :, bass.ts(j, CW)])
                packs.append(t)
            hst_tiles[j] = packs

        _load_hst_chunk(0)
        for c in range(1, NCB):
            _load_wc(c)

        cosT = tabs.tile([P, S], table_dt)
        nc.sync.dma_start(cosT[:], cosT_d[:, :])
        ssinT = tabs.tile([P, S], table_dt)
        nc.sync.dma_start(ssinT[:], ssinT_d[:, :])
        dmask = tabs.tile([P, CW // P, CW], act_dt)
        nc.sync.dma_start(dmask[:], dmask_d[:, :, :])

        # resident activations: qT (4 heads) + kT in one tile; v natural; outT
        qkT = res.tile([P, HL + 1, S], act_dt)
        v_sb = res.tile([P, NSQ, P], act_dt)
        outT = res.tile([P, HL, S], act_dt)

        # ---- phase 1: projections + RoPE + v transpose ----
        for j in range(NCH):
            jsl = bass.ts(j, CW)
            if j + 1 < NCH:
                _load_hst_chunk(j + 1)
            packs = hst_tiles.pop(j)
            for c in range(NCB):
                ps = psB.tile([P, CW], F32, tag="acc")
                for k in range(NKT):
                    nc.tensor.matmul(
                        ps[:], w_all[:, c, k * P:(k + 1) * P],
                        packs[k // KP][:, k % KP, :],
                        start=(k == 0), stop=(k == NKT - 1))
                if c < HL + 1:  # q heads and k: RoPE then store
                    raw = evq.tile([P, CW], act_dt, tag="raw")
                    nc.scalar.copy(raw[:], ps[:])
                    t1 = rope.tile([P, CW], act_dt, tag="t1")
                    nc.vector.tensor_mul(t1[:], raw[:], cosT[:, jsl])
                    t2 = rope.tile([P, CW], act_dt, tag="t2")
                    nc.vector.stream_shuffle(t2[:], raw[:], SWAP_MASK)
                    t3 = rope.tile([P, CW], act_dt, tag="t3")
                    nc.vector.tensor_mul(t3[:], t2[:], ssinT[:, jsl])
                    nc.vector.tensor_add(qkT[:, c, jsl], t1[:], t3[:])
                else:  # v: evict then PE-transpose into natural layout
                    vt = vtmp.tile([P, CW], F32, tag="vt")
                    nc.scalar.copy(vt[:], ps[:])
                    for b in range(CW // P):
                        pt = psM.tile([P, P], F32, tag="m")
                        nc.tensor.transpose(pt[:], vt[:, b * P:(b + 1) * P], ident[:])
                        nc.vector.tensor_copy(v_sb[:, j * (CW // P) + b, :], pt[:])

        # ---- phase 2: attention (scoresT layout; causal) ----
        for h in range(HL):
            for j in range(NCH):
                jsl = bass.ts(j, CW)
                nblk = (j + 1) * (CW // P)
                po = psO.tile([P, CW], F32, tag="po")
                pr = psM.tile([1, CW], F32, tag="m")
                for sk in range(nblk):
                    t = sk - j * (CW // P)
                    # diagonal tiles only need sq >= t*128 columns (rest is
                    # fully masked); trim the moving dim accordingly.
                    o = t * P if t > 0 else 0
                    csl = bass.ds(j * CW + o, CW - o)
                    pss = psB.tile([P, CW], F32, tag="acc")
                    nc.tensor.matmul(
                        pss[:, o:], qkT[:, HL, sk * P:(sk + 1) * P], qkT[:, h, csl],
                        start=True, stop=True)
                    pb = probs.tile([P, CW], act_dt, tag="pb")
                    nc.scalar.activation(
                        pb[:, o:], pss[:, o:], mybir.ActivationFunctionType.Exp,
                        bias=zbias[:], scale=SCALE)
                    if t >= 0:  # diagonal block: zero out sq < sk entries
                        nc.vector.tensor_mul(pb[:, o:], pb[:, o:], dmask[:, t, o:])
                    nc.tensor.matmul(po[:, o:], v_sb[:, sk, :], pb[:, o:],
                                     start=(sk == 0), stop=(sk == nblk - 1))
                    nc.tensor.matmul(pr[:, o:], ones_t[:], pb[:, o:],
                                     start=(sk == 0), stop=(sk == nblk - 1))
                # free the rowsum PSUM bank fast, then normalize off PE path
                rs = norm.tile([1, CW], F32, tag="rs")
                nc.scalar.copy(rs[:], pr[:])
                rc = norm.tile([1, CW], F32, tag="rc")
                sc = norm.tile([1, CW], F32, tag="sc")
                nc.vector.reciprocal_approx_accurate(rc[:], rs[:], sc[:])
                rb = norm.tile([P, CW], F32, tag="rb")
                nc.gpsimd.partition_broadcast(rb[:], rc[:], channels=P)
                nc.vector.tensor_mul(outT[:, h, jsl], po[:], rb[:])

        # ---- phase 3: wo projection ----
        for hc in range(NHC):
            wts = []
            for c in range(HL):
                wt = wow.tile([P, WOC], act_dt, tag="wt")
                nc.sync.dma_start(wt[:], wo_d[c, :, hc, :])
                wts.append(wt)
            for sq in range(NSQ):
                pf = psB.tile([P, WOC], F32, tag="acc")
                for c in range(HL):
                    nc.tensor.matmul(pf[:], outT[:, c, sq * P:(sq + 1) * P],
                                     wts[c][:], start=(c == 0), stop=(c == HL - 1))
                ob = outsb.tile([P, WOC], F32, tag="ob")
                if sq % 2 == 0:
                    nc.vector.tensor_copy(ob[:], pf[:])
                else:
                    nc.scalar.copy(ob[:], pf[:])
                nc.gpsimd.dma_start(out_d[sq * P:(sq + 1) * P, hc * WOC:(hc + 1) * WOC], ob[:])

    nc.compile()
    return nc


def _rope_tables():
    inv_freq = (1.0 / (BASE ** (np.arange(0, D, 2, dtype=np.float32) / D))).astype(np.float32)
    pos = np.arange(S, dtype=np.float32)[:, None]
    ang = pos * inv_freq[None, :]              # [S, D/2]
    sin = np.sin(ang).astype(np.float32).T     # [D/2, S]
    cos = np.cos(ang).astype(np.float32).T
    cosT = np.empty((D, S), np.float32)
    cosT[0::2] = cos
    cosT[1::2] = cos
    ssinT = np.empty((D, S), np.float32)
    ssinT[0::2] = -sin
    ssinT[1::2] = sin
    return cosT, ssinT


def _diag_masks():
    # dmask[p, t, f] = 1 where the scoreT element (sk=128t+p, sq=f) is causal-valid
    p = np.arange(P)[:, None, None]
    t = np.arange(CW // P)[None, :, None]
    f = np.arange(CW)[None, None, :]
    return (f >= P * t + p).astype(np.float32)


_NC_CACHE = {}


def kernel(hidden_states, wq, wk, wv, wo):
    global LAST_RESULT
    act_np = ml_dtypes.bfloat16
    key = "bf16"
    if key not in _NC_CACHE:
        _NC_CACHE[key] = _build()
    nc = _NC_CACHE[key]

    hs = np.asarray(hidden_states, np.float32).reshape(S, HID)
    hsT = np.ascontiguousarray(hs.T).astype(act_np)
    cosT, ssinT = _rope_tables()
    dmask = _diag_masks().astype(act_np)

    in_maps = []
    for i in range(NCORES):
        wqkv = np.concatenate(
            [np.asarray(wq, np.float32)[:, i * HL * D:(i + 1) * HL * D],
             np.asarray(wk, np.float32)[:, i * D:(i + 1) * D],
             np.asarray(wv, np.float32)[:, i * D:(i + 1) * D]], axis=1)
        # [HID, 768] -> [NCB, P, NKT*P]: block c, hid-in-tile p, (k-tile, col)
        wqkv = np.ascontiguousarray(
            wqkv.reshape(NKT, P, NCB, P).transpose(2, 1, 0, 3).reshape(NCB, P, NKT * P)
        ).astype(act_np)
        wo_i = np.ascontiguousarray(
            np.asarray(wo, np.float32)[i * HL * D:(i + 1) * HL * D, :]
            .reshape(HL, P, NHC, WOC)).astype(act_np)
        in_maps.append({
            "hsT": hsT, "wqkv": wqkv, "wo": wo_i,
            "cosT": cosT, "ssinT": ssinT, "dmask": dmask,
        })

    trace = bool(os.environ.get("BASS_KERNEL_TRACE"))
    res = run_bass_kernel_spmd(nc, in_maps, list(range(NCORES)),
                               trace=trace, trace_cores=[0] if trace else None)
    LAST_RESULT = res
    acc = np.zeros((S, HID), np.float32)
    for i in range(NCORES):
        acc += np.asarray(res.results[i]["out"], np.float32)
    return acc.reshape(B, S, HID)


# revision 12
# speedup vs baseline: 1.2632x; 1.0536x over previous
"""Trainium2 Bass kernel for Ernie4.5 attention (B=1, S=2048, HID=4096, H=32,
KVH=8, D=128), tensor-parallel over heads across 8 NeuronCores.

Core i owns q-heads 4i..4i+3, kv-head i, and wo rows [512*i, 512*(i+1)).
Each core computes its partial output [S, HID]; the host sums the 8 partials.

Per-core pipeline (all in transposed [feature, seq] layouts so no on-chip
transposition of activations is ever needed):
  1. qT/kT/vT = (w.T @ hsT-chunks) with weights stationary   -> [D, S] tiles
  2. RoPE on qT/kT via stream_shuffle (even/odd partition swap) + host tables
  3. per head: scoresT[sk,sq] = kT.T @ qT ; probsT = exp(scale*scoresT) with
     causal masking via precomputed 0/1 diag masks; row-sums via ones-matmul;
     outT[d,sq] += v[sk,d].T @ probsT accumulated in PSUM; normalize by recip
     row-sums (gpsimd partition_broadcast + vector mul)
  4. final[sq,hid] = sum_c outT[c].T @ wo[c], DMA'd straight to DRAM
"""

import os
import sys
from contextlib import ExitStack

import numpy as np

for _p in ("/opt/trn_rl_repo",):
    if os.path.isdir(_p) and _p not in sys.path:
        sys.path.append(_p)

import ml_dtypes

import concourse.bass as bass
import concourse.mybir as mybir
import concourse.tile as tile
from concourse import bacc
from concourse.bass_utils import run_bass_kernel_spmd
from concourse.masks import make_identity

P = 128
B, S, HID, H, KVH, D = 1, 2048, 4096, 32, 8, 128
NCORES = 8
HL = H // NCORES          # 4 local q heads
NKT = HID // P            # 32 contraction tiles
NSQ = S // P              # 16 seq blocks
CW = 512                  # seq chunk width
NCH = S // CW             # 4 seq chunks
KP = 4                    # hsT k-tiles packed per DMA
WOC = 512                 # wo output chunk width
NHC = HID // WOC          # 8 wo output chunks
NCB = HL + 2              # 6 projection column blocks (4 q heads, k, v)
SCALE = float(D) ** -0.5
BASE = 10000.0

F32 = mybir.dt.float32
BF16 = mybir.dt.bfloat16
SWAP_MASK = [i ^ 1 for i in range(32)]

LAST_RESULT = None


def _build(act_dt=BF16, table_dt=F32):
    """Emit the SPMD per-core program. act_dt = matmul operand dtype."""
    nc = bacc.Bacc("TRN2", target_bir_lowering=False, debug=False)

    hsT_d = nc.dram_tensor("hsT", [HID, S], act_dt, kind="ExternalInput").ap()
    wqkv_d = nc.dram_tensor("wqkv", [NCB, P, NKT * P], act_dt, kind="ExternalInput").ap()
    wo_d = nc.dram_tensor("wo", [HL, P, NHC, WOC], act_dt, kind="ExternalInput").ap()
    cosT_d = nc.dram_tensor("cosT", [P, S], table_dt, kind="ExternalInput").ap()
    ssinT_d = nc.dram_tensor("ssinT", [P, S], table_dt, kind="ExternalInput").ap()
    dmask_d = nc.dram_tensor("dmask", [P, CW // P, CW], act_dt, kind="ExternalInput").ap()
    out_d = nc.dram_tensor("out", [S, HID], F32, kind="ExternalOutput").ap()

    with tile.TileContext(nc) as tc, ExitStack() as ctx:
        const = ctx.enter_context(tc.tile_pool(name="const", bufs=1))
        wpool = ctx.enter_context(tc.tile_pool(name="wpool", bufs=1))
        tabs = ctx.enter_context(tc.tile_pool(name="tabs", bufs=1))
        res = ctx.enter_context(tc.tile_pool(name="res", bufs=1))
        hst = ctx.enter_context(tc.tile_pool(name="hst", bufs=NKT // KP + 1))
        evq = ctx.enter_context(tc.tile_pool(name="evq", bufs=3))
        rope = ctx.enter_context(tc.tile_pool(name="rope", bufs=3))
        vtmp = ctx.enter_context(tc.tile_pool(name="vtmp", bufs=2))
        probs = ctx.enter_context(tc.tile_pool(name="probs", bufs=10))
        norm = ctx.enter_context(tc.tile_pool(name="norm", bufs=2))
        wow = ctx.enter_context(tc.tile_pool(name="wow", bufs=8))
        outsb = ctx.enter_context(tc.tile_pool(name="outsb", bufs=6))
        # PSUM: 8 banks total. "big" [128,512]f32 slots shared (same tag)
        # across proj-acc, scores, and wo-acc; psO holds attention outT
        # accumulators; psM shared by v-transpose and rowsums.
        psB = ctx.enter_context(tc.tile_pool(name="psB", bufs=4, space="PSUM"))
        psO = ctx.enter_context(tc.tile_pool(name="psO", bufs=2, space="PSUM"))
        psM = ctx.enter_context(tc.tile_pool(name="psM", bufs=2, space="PSUM"))

        ones_t = const.tile([P, 1], act_dt)
        nc.vector.memset(ones_t[:], 1.0)
        ident = const.tile([P, P], F32)
        make_identity(nc, ident[:])
        zbias = const.tile([P, 1], F32)
        nc.vector.memset(zbias[:], 0.0)

        # weights: one resident tile + one DMA per column block. c=0 first so
        # the first matmuls' inputs land early; the rest follow the first hsT
        # chunk in queue order.
        w_all = wpool.tile([P, NCB, NKT * P], act_dt)

        def _load_wc(c):
            nc.sync.dma_start(w_all[:, c, :], wqkv_d[c, :, :])

        # first k-tile of c=0 lands first so the very first matmul can start
        nc.sync.dma_start(w_all[:, 0, :P], wqkv_d[0, :, :P])

        hst_tiles = {}

        def _load_hst_chunk(j):
            packs = []
            for g in range(NKT // KP):
                t = hst.tile([P, KP, CW], act_dt, tag="hst")
                nc.sync.dma_start(
                    t[:], hsT_d.rearrange("(g kp p) s -> g p kp s", g=NKT // KP, kp=KP, p=P)[
                        g, :, :, bass.ts(j, CW)])
                packs.append(t)
            hst_tiles[j] = packs

        _load_hst_chunk(0)
        nc.sync.dma_start(w_all[:, 0, P:], wqkv_d[0, :, P:])
        for c in range(1, NCB):
            _load_wc(c)

        cosT = tabs.tile([P, S], table_dt)
        nc.sync.dma_start(cosT[:], cosT_d[:, :])
        ssinT = tabs.tile([P, S], table_dt)
        nc.sync.dma_start(ssinT[:], ssinT_d[:, :])
        dmask = tabs.tile([P, CW // P, CW], act_dt)
        nc.sync.dma_start(dmask[:], dmask_d[:, :, :])

        # resident activations: qT (4 heads) + kT in one tile; v natural; outT
        qkT = res.tile([P, HL + 1, S], act_dt)
        v_sb = res.tile([P, NSQ, P], act_dt)
        outT = res.tile([P, HL, S], act_dt)

        # ---- phase 1: projections + RoPE + v transpose ----
        for j in range(NCH):
            jsl = bass.ts(j, CW)
            if j + 1 < NCH:
                _load_hst_chunk(j + 1)
            packs = hst_tiles.pop(j)
            for c in range(NCB):
                ps = psB.tile([P, CW], F32, tag="acc")
                for k in range(NKT):
                    nc.tensor.matmul(
                        ps[:], w_all[:, c, k * P:(k + 1) * P],
                        packs[k // KP][:, k % KP, :],
                        start=(k == 0), stop=(k == NKT - 1))
                if c < HL + 1:  # q heads and k: RoPE then store
                    raw = evq.tile([P, CW], act_dt, tag="raw")
                    nc.scalar.copy(raw[:], ps[:])
                    t1 = rope.tile([P, CW], act_dt, tag="t1")
                    nc.vector.tensor_mul(t1[:], raw[:], cosT[:, jsl])
                    t2 = rope.tile([P, CW], act_dt, tag="t2")
                    nc.vector.stream_shuffle(t2[:], raw[:], SWAP_MASK)
                    t3 = rope.tile([P, CW], act_dt, tag="t3")
                    nc.vector.tensor_mul(t3[:], t2[:], ssinT[:, jsl])
                    nc.vector.tensor_add(qkT[:, c, jsl], t1[:], t3[:])
                else:  # v: evict then PE-transpose into natural layout
                    vt = vtmp.tile([P, CW], F32, tag="vt")
                    nc.scalar.copy(vt[:], ps[:])
                    for b in range(CW // P):
                        pt = psM.tile([P, P], F32, tag="m")
                        nc.tensor.transpose(pt[:], vt[:, b * P:(b + 1) * P], ident[:])
                        nc.vector.tensor_copy(v_sb[:, j * (CW // P) + b, :], pt[:])

        # ---- phase 2: attention (scoresT layout; causal) ----
        for h in range(HL):
            for j in range(NCH):
                jsl = bass.ts(j, CW)
                nblk = (j + 1) * (CW // P)
                po = psO.tile([P, CW], F32, tag="po")
                pr = psM.tile([1, CW], F32, tag="m")
                for sk in range(nblk):
                    t = sk - j * (CW // P)
                    # diagonal tiles only need sq >= t*128 columns (rest is
                    # fully masked); trim the moving dim accordingly.
                    o = t * P if t > 0 else 0
                    csl = bass.ds(j * CW + o, CW - o)
                    pss = psB.tile([P, CW], F32, tag="acc")
                    nc.tensor.matmul(
                        pss[:, o:], qkT[:, HL, sk * P:(sk + 1) * P], qkT[:, h, csl],
                        start=True, stop=True)
                    pb = probs.tile([P, CW], act_dt, tag="pb")
                    nc.scalar.activation(
                        pb[:, o:], pss[:, o:], mybir.ActivationFunctionType.Exp,
                        bias=zbias[:], scale=SCALE)
                    if t >= 0:  # diagonal block: zero out sq < sk entries
                        nc.vector.tensor_mul(pb[:, o:], pb[:, o:], dmask[:, t, o:])
                    nc.tensor.matmul(po[:, o:], v_sb[:, sk, :], pb[:, o:],
                                     start=(sk == 0), stop=(sk == nblk - 1))
                    nc.tensor.matmul(pr[:, o:], ones_t[:], pb[:, o:],
                                     start=(sk == 0), stop=(sk == nblk - 1))
                # free the rowsum PSUM bank fast, then normalize off PE path
                rs = norm.tile([1, CW], F32, tag="rs")
                nc.scalar.copy(rs[:], pr[:])
                rc = norm.tile([1, CW], F32, tag="rc")
                sc = norm.tile([1, CW], F32, tag="sc")
                nc.vector.reciprocal_approx_accurate(rc[:], rs[:], sc[:])
                rb = norm.tile([P, CW], F32, tag="rb")
                nc.gpsimd.partition_broadcast(rb[:], rc[:], channels=P)
                nc.vector.tensor_mul(outT[:, h, jsl], po[:], rb[:])

        # ---- phase 3: wo projection ----
        for hc in range(NHC):
            wts = []
            for c in range(HL):
                wt = wow.tile([P, WOC], act_dt, tag="wt")
                nc.sync.dma_start(wt[:], wo_d[c, :, hc, :])
                wts.append(wt)
            for sq in range(NSQ):
                pf = psB.tile([P, WOC], F32, tag="acc")
                for c in range(HL):
                    nc.tensor.matmul(pf[:], outT[:, c, sq * P:(sq + 1) * P],
                                     wts[c][:], start=(c == 0), stop=(c == HL - 1))
                ob = outsb.tile([P, WOC], F32, tag="ob")
                if sq % 2 == 0:
                    nc.vector.tensor_copy(ob[:], pf[:])
                else:
                    nc.scalar.copy(ob[:], pf[:])
                nc.gpsimd.dma_start(out_d[sq * P:(sq + 1) * P, hc * WOC:(hc + 1) * WOC], ob[:])

    nc.compile()
    return nc


def _rope_tables():
    inv_freq = (1.0 / (BASE ** (np.arange(0, D, 2, dtype=np.float32) / D))).astype(np.float32)
    pos = np.arange(S, dtype=np.float32)[:, None]
    ang = pos * inv_freq[None, :]              # [S, D/2]
    sin = np.sin(ang).astype(np.float32).T     # [D/2, S]
    cos = np.cos(ang).astype(np.float32).T
    cosT = np.empty((D, S), np.float32)
    cosT[0::2] = cos
    cosT[1::2] = cos
    ssinT = np.empty((D, S), np.float32)
    ssinT[0::2] = -sin
    ssinT[1::2] = sin
    return cosT, ssinT


def _diag_masks():
    # dmask[p, t, f] = 1 where the scoreT element (sk=128t+p, sq=f) is causal-valid
    p = np.arange(P)[:, None, None]
    t = np.arange(CW // P)[None, :, None]
    f = np.arange(CW)[None, None, :]
    return (f >= P * t + p).astype(np.float32)


_NC_CACHE = {}


def kernel(hidden_states, wq, wk, wv, wo):
    global LAST_RESULT
    act_np = ml_dtypes.bfloat16
    key = "bf16"
    if key not in _NC_CACHE:
        _NC_CACHE[key] = _build()
    nc = _NC_CACHE[key]

    hs = np.asarray(hidden_states, np.float32).reshape(S, HID)
    hsT = np.ascontiguousarray(hs.T).astype(act_np)
    cosT, ssinT = _rope_tables()
    dmask = _diag_masks().astype(act_np)

    in_maps = []
    for i in range(NCORES):
        wqkv = np.concatenate(
            [np.asarray(wq, np.float32)[:, i * HL * D:(i + 1) * HL * D],
             np.asarray(wk, np.float32)[:, i * D:(i + 1) * D],
             np.asarray(wv, np.float32)[:, i * D:(i + 1) * D]], axis=1)
        # [HID, 768] -> [NCB, P, NKT*P]: block c, hid-in-tile p, (k-tile, col)
        wqkv = np.ascontiguousarray(
            wqkv.reshape(NKT, P, NCB, P).transpose(2, 1, 0, 3).reshape(NCB, P, NKT * P)
        ).astype(act_np)
        wo_i = np.ascontiguousarray(
            np.asarray(wo, np.float32)[i * HL * D:(i + 1) * HL * D, :]
            .reshape(HL, P, NHC, WOC)).astype(act_np)
        in_maps.append({
            "hsT": hsT, "wqkv": wqkv, "wo": wo_i,
            "cosT": cosT, "ssinT": ssinT, "dmask": dmask,
        })

    trace = bool(os.environ.get("BASS_KERNEL_TRACE"))
    res = run_bass_kernel_spmd(nc, in_maps, list(range(NCORES)),
                               trace=trace, trace_cores=[0] if trace else None)
    LAST_RESULT = res
    acc = np.zeros((S, HID), np.float32)
    for i in range(NCORES):
        acc += np.asarray(res.results[i]["out"], np.float32)
    return acc.reshape(B, S, HID)
